# revision 11
# baseline (speedup 1.0000x reference)
"""Causal single-head attention (B=16, S=2048, D=1024, HD=64) on 8 TRN2 cores.

Data-parallel: 2 batches per core. Per batch, streamed over 4 seq-tiles of 512:
  - PE-transpose X tiles to X^T (contraction dim must sit on partitions),
  - two packed projection passes: stationary [Wk|Wq] and [Wq|Wv] over X^T
    (fp32r matmuls, N=512 moving -> full PE rate, fp32 PSUM accumulate),
  - S^T = K @ Q^T per (kseq-block, qtile) with causal block skipping and
    diagonal narrowing; exp on ScalarE with the 1/sqrt(64) scale folded in,
  - O^T accumulation with stationary [V | ones]: the ones column yields the
    softmax denominators for free,
  - PE-transpose O^T back to natural layout, multiply by 1/sum, DMA out.
"""
import numpy as np

import concourse.bacc as bacc
import concourse.mybir as mybir
import concourse.tile as tile
from concourse import bass_utils

B, S, D, HD = 16, 2048, 1024, 64
N_CORES = 8
BPC = B // N_CORES          # batches per core
ST = 512                    # seq tile (qtile) size
NST = S // ST               # 4 seq tiles per batch
NDB = D // 128              # 8 d-blocks
NKB = S // 128              # 16 kseq blocks per batch

f32 = mybir.dt.float32
f32r = mybir.dt.float32r

_cache = {}


def _build():
    nc = bacc.Bacc("TRN2", target_bir_lowering=False, debug=False,
                   num_devices=N_CORES)

    x = nc.dram_tensor("x", [BPC, S, D], f32, kind="ExternalInput")
    # host-packed constants: identity | mask | [Wk|Wq] | [Wq|Wv] | biases | ones
    consts_d = nc.dram_tensor("consts_d", [128, 2306 + NKB], f32,
                              kind="ExternalInput")
    out = nc.dram_tensor("out", [BPC, S, HD], f32, kind="ExternalOutput")

    with tile.TileContext(nc) as tc:
        with (
            tc.tile_pool(name="consts", bufs=1) as consts,
            tc.tile_pool(name="big", bufs=2) as big,
            tc.tile_pool(name="perbatch", bufs=2) as perbatch,
            tc.tile_pool(name="work", bufs=3) as work,
            tc.tile_pool(name="pp", bufs=1, space="PSUM") as pp,
            tc.tile_pool(name="ps_t", bufs=2, space="PSUM") as ps_t,
            tc.tile_pool(name="ps_s", bufs=2, space="PSUM") as ps_s,
            tc.tile_pool(name="ps_o", bufs=1, space="PSUM") as ps_o,
            tc.tile_pool(name="ps_m", bufs=1, space="PSUM") as ps_m,
        ):
            cst = consts.tile([128, 2306 + NKB], f32r)
            # identity+mask land first so transposes can start immediately
            nc.sync.dma_start(out=cst[:, 0:256],
                              in_=consts_d.ap()[:, 0:256].bitcast(f32r))
            nc.sync.dma_start(out=cst[:, 256:],
                              in_=consts_d.ap()[:, 256:].bitcast(f32r))
            identr = cst[:, 0:128]
            ident = identr.bitcast(f32)
            mask = cst[:, 128:256]
            wkq = cst[:, 256:1280].rearrange("p (db m) -> p db m", db=NDB)
            wqv = cst[:, 1280:2304].rearrange("p (db m) -> p db m", db=NDB)
            bias_kq = cst.bitcast(f32)[:, 2304:2305]
            bias_qv = cst.bitcast(f32)[:, 2305:2306]
            ones_c = cst[:, 2306:2306 + NKB]

            for b in range(BPC):
                # K^T per seqtile (rows 0:64; rows 64:128 hold an unused Q^T copy)
                kq_sb = perbatch.tile([128, NST, ST], f32r)
                # V natural with ones column: [128, kb, 65]
                vn_sb = perbatch.tile([128, NKB, 65], f32r)
                nc.gpsimd.tensor_copy(
                    out=vn_sb[:, :, 64:65],
                    in_=ones_c.rearrange("p (t o) -> p t o", o=1))

                for st in range(NST):
                    # ---- load X natural, transpose to X^T ----
                    xn = big.tile([128, NST, D], f32r, tag="xn")
                    for t in range(4):
                        nc.sync.dma_start(
                            out=xn[:, t, :],
                            in_=x.ap()[b, ST * st + 128 * t:ST * st + 128 * (t + 1), :]
                            .bitcast(f32r))
                    xt_sb = big.tile([128, NDB, ST], f32r, tag="xt")
                    for db in range(NDB):
                        xt_ps = ps_t.tile([128, ST], f32r)
                        for t in range(4):
                            nc.tensor.transpose(
                                xt_ps[:, 128 * t:128 * (t + 1)],
                                xn[:, t, 128 * db:128 * (db + 1)],
                                identr)
                        nc.vector.tensor_copy(out=xt_sb[:, db, :], in_=xt_ps)

                    # ---- projections ----
                    p1 = pp.tile([128, ST], f32, tag="p1")
                    p2 = pp.tile([128, ST], f32, tag="p2")
                    for db in range(NDB):
                        nc.tensor.matmul(p1, wkq[:, db, :], xt_sb[:, db, :],
                                         start=(db == 0), stop=(db == NDB - 1))
                        nc.tensor.matmul(p2, wqv[:, db, :], xt_sb[:, db, :],
                                         start=(db == 0), stop=(db == NDB - 1))
                    nc.scalar.activation(out=kq_sb[:, st, :], in_=p1,
                                         func=mybir.ActivationFunctionType.Identity,
                                         bias=bias_kq)
                    qv_sb = work.tile([128, ST], f32r, tag="qv")
                    nc.scalar.activation(out=qv_sb, in_=p2,
                                         func=mybir.ActivationFunctionType.Identity,
                                         bias=bias_qv)

                    # ---- V natural (transpose V^T blocks, packed into one bank) ----
                    vn_ps = ps_m.tile([128, 4, 64], f32r, tag="tp")
                    for c in range(4):
                        nc.tensor.transpose(
                            vn_ps[:, c, :],
                            qv_sb[64:128, 128 * c:128 * (c + 1)],
                            identr[64:128, 64:128])
                    nc.vector.tensor_copy(
                        out=vn_sb[:, 4 * st:4 * st + 4, 0:64], in_=vn_ps)

                    # ---- attention for qtile st ----
                    o_ps = ps_o.tile([65, ST], f32)
                    kb_last = 4 * st + 3
                    for kb in range(4 * st + 4):
                        j = kb - 4 * st
                        if j < 0:
                            w, qoff = ST, 0
                        else:
                            w, qoff = ST - 128 * j, 128 * j
                        s_ps = ps_s.tile([128, ST], f32)
                        nc.tensor.matmul(
                            s_ps[:, 0:w],
                            kq_sb[0:64, kb // 4, 128 * (kb % 4):128 * (kb % 4) + 128],
                            qv_sb[0:64, qoff:qoff + w],
                            start=True, stop=True)
                        e_sb = work.tile([128, ST], f32r, tag="e", bufs=4)
                        nc.scalar.activation(out=e_sb[:, 0:w], in_=s_ps[:, 0:w],
                                             func=mybir.ActivationFunctionType.Exp,
                                             scale=float(HD) ** -0.5)
                        if j >= 0:
                            nc.vector.tensor_mul(out=e_sb[:, 0:128],
                                                 in0=e_sb[:, 0:128], in1=mask)
                        nc.tensor.matmul(
                            o_ps[:, qoff:qoff + w],
                            vn_sb[:, kb, :],
                            e_sb[:, 0:w],
                            start=(kb == 0), stop=(kb == kb_last))

                    # ---- finalize: transpose O^T, normalize, store ----
                    ot_sb = work.tile([65, ST], f32, tag="ot")
                    nc.scalar.copy(out=ot_sb, in_=o_ps)
                    o_qt = work.tile([128, 4, HD], f32, tag="oq")
                    for c in range(4):
                        on_ps = ps_m.tile([128, 65], f32, tag="tp")
                        nc.tensor.transpose(
                            on_ps,
                            ot_sb[:, 128 * c:128 * (c + 1)],
                            ident[0:65, 0:65])
                        recip = work.tile([128, 1], f32, tag="rc")
                        nc.vector.reciprocal(out=recip, in_=on_ps[:, 64:65])
                        nc.vector.tensor_scalar_mul(
                            out=o_qt[:, c, :], in0=on_ps[:, 0:64], scalar1=recip)
                    nc.sync.dma_start(
                        out=out.ap()[b, ST * st:ST * (st + 1), :]
                        .rearrange("(t p) d -> p t d", p=128),
                        in_=o_qt)

    nc.compile()
    return nc


def _pack_consts(wq, wk, wv, bq, bk, bv):
    """[128, 2306+NKB]: identity | causal mask | [Wk|Wq] | [Wq|Wv] | bias cols | ones."""
    def packed_pair(wa, wb):
        pa = wa.reshape(NDB, 128, HD).transpose(1, 0, 2)   # [128, db, 64]
        pb = wb.reshape(NDB, 128, HD).transpose(1, 0, 2)
        return np.concatenate([pa, pb], axis=2).reshape(128, NDB * 128)

    cst = np.zeros((128, 2306 + NKB), dtype=np.float32)
    cst[:, 0:128] = np.eye(128, dtype=np.float32)
    cst[:, 128:256] = (np.arange(128)[None, :] >= np.arange(128)[:, None])
    cst[:, 256:1280] = packed_pair(wk, wq)
    cst[:, 1280:2304] = packed_pair(wq, wv)
    cst[:, 2304] = np.concatenate([bk, bq])
    cst[:, 2305] = np.concatenate([bq, bv])
    cst[:, 2306:] = 1.0
    return np.ascontiguousarray(cst)


def kernel(x, Wq, bq, Wk, bk, Wv, bv):
    if "nc" not in _cache:
        _cache["nc"] = _build()
    nc = _cache["nc"]

    x = np.ascontiguousarray(np.asarray(x, dtype=np.float32))
    cst = _pack_consts(np.asarray(Wq, np.float32), np.asarray(Wk, np.float32),
                       np.asarray(Wv, np.float32), np.asarray(bq, np.float32),
                       np.asarray(bk, np.float32), np.asarray(bv, np.float32))

    in_maps = []
    for c in range(N_CORES):
        in_maps.append({
            "x": x[c * BPC:(c + 1) * BPC],
            "consts_d": cst,
        })

    res = bass_utils.run_bass_kernel_spmd(nc, in_maps,
                                          core_ids=list(range(N_CORES)),
                                          **_cache.get("run_kwargs", {}))
    _cache["last_result"] = res
    return np.concatenate([res.results[c]["out"] for c in range(N_CORES)], axis=0)
